# revision 28
# baseline (speedup 1.0000x reference)
"""Trainium2 Bass kernel for the DSSM (dual-modality Mamba-style 2D selective
scan) module. 8-core SPMD: scan channels d-sharded (24/core x 4 directions),
upstream in_proj/dwconv d-sharded, downstream LN/out position-sharded.
Cross-core: 3 chunked AllReduces (x_dbl partials, one per scan tile,
overlapped with compute) + tiny v1 AllReduce + one AllToAll (y reshard).
"""
import sys
sys.path.insert(0, "/opt/trn_rl_repo")
import numpy as np
import ml_dtypes
import concourse.bass as bass
from concourse import mybir
from concourse.bacc import Bacc
from concourse.tile import TileContext
from concourse.tile_rust import add_dep_helper
from concourse.bass_utils import run_bass_kernel_spmd

F32 = mybir.dt.float32
F32R = mybir.dt.float32r
BF16 = mybir.dt.bfloat16
AF = mybir.ActivationFunctionType
OP = mybir.AluOpType

NCORES = 8
RG = [list(range(NCORES))]
B, H, W = 1, 48, 48
HW = H * W                      # 2304
L = 2 * HW                      # 4608
DM = 96                         # d_model
DI = 192                        # d_inner
NST = 4                         # d_state
RNK = 6                         # dt_rank
K = 4
DSL = DI // NCORES              # 24 channels per core
LANES = NST * DSL               # 96 scan lanes (lane = n*DSL + d)
CH = 512                        # phase-B PSUM column chunk
NCH = L // CH                   # 9
PC = HW // NCORES               # 288 positions per core (phase C)
RCH = 480                       # phase-A chunk = 10 image rows
ROWCHUNKS = [(0, 10), (10, 10), (20, 10), (30, 10), (40, 8)]
XOFF = {"sub": 0, "vi": 32, "ir": 64}   # row block in stacked xs96
# tile t: (half0 mod, half1 mod); k per segment = t (t<2) else 2/3
TMODS = (("sub", "vi"), ("sub", "ir"), ("vi", "ir"))

_cache = {}


def _build():
    nc = Bacc(trn_type="TRN2", num_devices=NCORES)
    EIn = dict(kind="ExternalInput")
    i_xvt = nc.dram_tensor("xvt", [DM, HW], F32, **EIn)
    i_xit = nc.dram_tensor("xit", [DM, HW], F32, **EIn)
    i_w48v = nc.dram_tensor("w48v", [DM, 64], F32, **EIn)  # [x@0|z@32] lhsT
    i_w48i = nc.dram_tensor("w48i", [DM, 64], F32, **EIn)  # [x@0|z@32]
    i_wsub = nc.dram_tensor("wsub", [DM, DSL], F32, **EIn)
    i_w72 = nc.dram_tensor("w72", [96, 9, 96], F32, **EIn)   # conv block-diag
    i_b72 = nc.dram_tensor("b72", [96, 1], F32, **EIn)       # conv bias stacked
    i_w84 = nc.dram_tensor("w84", [96, 3, 28], BF16, **EIn)  # x_dbl per tile
    i_wdtr = nc.dram_tensor("wdtr", [RNK, K, LANES], BF16, **EIn)
    i_dtb = nc.dram_tensor("dtb", [LANES, K], F32, **EIn)
    i_asc = nc.dram_tensor("asc", [LANES, K], F32, **EIn)
    i_m96 = nc.dram_tensor("m96", [LANES, DSL], F32, **EIn)
    i_diagd = nc.dram_tensor("diagd", [96, 2, DSL], BF16, **EIn)  # (vi,ir) D
    i_f1 = nc.dram_tensor("f1", [DSL, 4, 12], F32, **EIn)   # (via,vim,ira,irm)
    i_f2 = nc.dram_tensor("f2", [12, 2, 2, DM], F32, **EIn)  # (mod, chunk, out)
    i_lnw = nc.dram_tensor("lnw", [DM, 2, 4], F32, **EIn)    # per chunk g/b
    i_wout = nc.dram_tensor("wout", [DM, 2, DM], F32, **EIn)
    i_wz = nc.dram_tensor("wz", [DM, 4, DM], F32, **EIn)     # z lhsT
    i_onec = nc.dram_tensor("onec", [DM, 1], F32, **EIn)
    i_oner = nc.dram_tensor("oner", [1, DM], F32, **EIn)
    i_xvc = nc.dram_tensor("xvc", [DM, PC], F32, **EIn)
    i_xic = nc.dram_tensor("xic", [DM, PC], F32, **EIn)
    o_out = nc.dram_tensor("out", [DM, PC], F32, kind="ExternalOutput")
    # collective DRAM buffers: per scan tile [half, row, col]
    d_ri = [nc.dram_tensor(f"d_ri{t}", [2, 14, HW], BF16) for t in range(3)]
    d_ro = [nc.dram_tensor(f"d_ro{t}", [2, 14, HW], BF16, addr_space="Shared")
            for t in range(3)]
    d_v1i = nc.dram_tensor("d_v1i", [12, 4], F32)
    d_v1o = nc.dram_tensor("d_v1o", [12, 4], F32, addr_space="Shared")
    d_a2i = nc.dram_tensor("d_a2i", [NCORES, 2 * DSL, PC], F32)
    d_a2o = nc.dram_tensor("d_a2o", [NCORES, 2 * DSL, PC], F32)

    def mmr(out, lhsT, rhs, **kw):
        # float32r matmul: 1 cycle/row (vs 4 for fp32) when free dim >= 256
        nc.tensor.matmul(out, lhsT.bitcast(F32R), rhs.bitcast(F32R), **kw)

    import contextlib
    with TileContext(nc) as tc, contextlib.ExitStack() as ctx:
        wpool = ctx.enter_context(tc.tile_pool(name="weights", bufs=1))
        big = ctx.enter_context(tc.tile_pool(name="big", bufs=1))

        def wtile(shape, src, rnd=False, dt=F32):
            t = wpool.tile(shape, dt, tag=src.name, name="w_" + src.name)
            if rnd:
                nc.sync.dma_start(out=t[:].bitcast(F32R),
                                  in_=src[:].bitcast(F32R))
            else:
                nc.sync.dma_start(out=t, in_=src[:])
            return t
        t_w48v = wtile([DM, 64], i_w48v, True)
        t_w48i = wtile([DM, 64], i_w48i, True)
        t_wsub = wtile([DM, DSL], i_wsub, True)
        t_w72 = wtile([96, 9, 96], i_w72, True)
        t_b72 = wtile([96, 1], i_b72)
        t_w84 = wtile([96, 3, 28], i_w84, dt=BF16)
        t_wdtr = wtile([RNK, K, LANES], i_wdtr, dt=BF16)
        t_dtb = wtile([LANES, K], i_dtb)
        t_asc = wtile([LANES, K], i_asc)
        t_m96 = wtile([LANES, DSL], i_m96, True)
        t_diagd = wtile([96, 2, DSL], i_diagd, dt=BF16)
        t_f1 = wtile([DSL, 4, 12], i_f1)
        t_f2 = wtile([12, 2, 2, DM], i_f2)
        t_lnw = wtile([DM, 2, 4], i_lnw)
        t_wout = wtile([DM, 2, DM], i_wout, True)
        t_wz = wtile([DM, 4, DM], i_wz, True)
        t_onec = wtile([DM, 1], i_onec)
        t_oner = wtile([1, DM], i_oner)
        t_xvc = wtile([DM, PC], i_xvc, True)
        t_xic = wtile([DM, PC], i_xic, True)

        # persistent SBUF
        t_xs72 = big.tile([96, HW], BF16, tag="xs72")  # (sub|vi|ir) @ 0/32/64
        t_yvi = big.tile([DSL, HW], F32, tag="yvi")
        t_yir = big.tile([DSL, HW], F32, tag="yir")

        # =========== PHASE A: upstream (d-sharded) ===========
        with tc.tile_pool(name="pa1", bufs=1) as pa1, \
             tc.tile_pool(name="pa", bufs=3) as pa, \
             tc.tile_pool(name="pap", bufs=2, space="PSUM") as pap, \
             tc.tile_pool(name="pas", bufs=2, space="PSUM") as pas, \
             tc.tile_pool(name="pav", bufs=2, space="PSUM") as pav, \
             tc.tile_pool(name="pav1", bufs=1, space="PSUM") as pav1:
            t_xvt = pa1.tile([DM, HW], F32, tag="xvt")
            nc.sync.dma_start(out=t_xvt[:].bitcast(F32R),
                              in_=i_xvt[:].bitcast(F32R))
            t_xit = pa1.tile([DM, HW], F32, tag="xit")
            nc.sync.dma_start(out=t_xit[:].bitcast(F32R),
                              in_=i_xit[:].bitcast(F32R))
            t_xdiff = pa1.tile([DM, HW], F32, tag="xdiff")
            nc.vector.tensor_sub(t_xdiff[:].bitcast(F32R), t_xvt[:], t_xit[:])

            pad72 = pa1.tile([96, 50, 50], F32, tag="pad72")
            nc.vector.memset(pad72[:], 0.0)

            # in_proj: merged [zv|xv] / [zi|xi] matmuls + sub
            t_zacc = pa1.tile([DSL, 2, len(ROWCHUNKS)], F32, tag="zacc")
            t_zc = {"vi": pa1.tile([DSL, HW], F32, tag="zcvi", name="zcvi"),
                    "ir": pa1.tile([DSL, HW], F32, tag="zcir", name="zcir")}
            for im, (mod, w48, xt) in enumerate(
                    (("vi", t_w48v, t_xvt), ("ir", t_w48i, t_xit))):
                for ic, (r0, nr) in enumerate(ROWCHUNKS):
                    cols = slice(r0 * W, (r0 + nr) * W)
                    p48 = pap.tile([64, RCH], F32, tag="p48")
                    mmr(p48[:, :nr * W], w48[:], xt[:, cols],
                        start=True, stop=True)
                    nc.scalar.activation(t_zc[mod][:, cols],
                                         p48[32:56, :nr * W], AF.Silu,
                                         accum_out=t_zacc[:, im, ic:ic + 1])
                    o = XOFF[mod]
                    nc.scalar.copy(
                        pad72[o:o + DSL, 1 + r0:1 + r0 + nr, 1:49]
                        .bitcast(F32R),
                        p48[0:DSL, :nr * W]
                        .rearrange("p (a b) -> p a b", a=nr))
            for (r0, nr) in ROWCHUNKS:
                cols = slice(r0 * W, (r0 + nr) * W)
                p24 = pas.tile([28, RCH], F32, tag="px")
                mmr(p24[0:DSL, :nr * W], t_wsub[:], t_xdiff[:, cols],
                    start=True, stop=True)
                nc.scalar.copy(
                    pad72[0:DSL, 1 + r0:1 + r0 + nr, 1:49].bitcast(F32R),
                    p24[0:DSL, :nr * W].rearrange("p (a b) -> p a b", a=nr))

            # chan-attn pooled stats -> v1 partials -> tiny AR
            t_pool = pa1.tile([DSL, 4], F32, tag="tpool")
            nc.vector.tensor_reduce(t_pool[:, 0:1], t_zacc[:, 0, :],
                                    axis=mybir.AxisListType.X, op=OP.add)
            nc.vector.tensor_reduce(t_pool[:, 1:2], t_zc["vi"][:],
                                    axis=mybir.AxisListType.X, op=OP.max)
            nc.vector.tensor_reduce(t_pool[:, 2:3], t_zacc[:, 1, :],
                                    axis=mybir.AxisListType.X, op=OP.add)
            nc.vector.tensor_reduce(t_pool[:, 3:4], t_zc["ir"][:],
                                    axis=mybir.AxisListType.X, op=OP.max)
            t_v1 = pa1.tile([12, 4], F32, tag="tv1")
            p_v1 = pav1.tile([12, 4], F32, tag="pv1")
            for j in range(4):
                nc.tensor.matmul(p_v1[:, j:j + 1], t_f1[:, j, :],
                                 t_pool[:, j:j + 1], start=True, stop=True)
            nc.scalar.copy(t_v1[:], p_v1[:])
            nc.sync.dma_start(out=d_v1i[:], in_=t_v1[:])

            # depthwise conv 3x3: block-diag 72-channel, 9 taps
            for (r0, nr) in ROWCHUNKS:
                p_c = pav.tile([96, RCH], F32, tag="pconv")
                for tap in range(9):
                    dy, dx = tap // 3, tap % 3
                    mmr(p_c[:, :nr * W], t_w72[:, tap, :],
                        pad72[:, r0 + dy:r0 + dy + nr, dx:dx + 48],
                        start=(tap == 0), stop=(tap == 8))
                nc.scalar.activation(
                    t_xs72[:, r0 * W:(r0 + nr) * W],
                    p_c[:, :nr * W], AF.Silu, bias=t_b72[:, 0:1], scale=1.0)

            # x_dbl partials per scan tile -> DRAM -> chunked AllReduce
            ar_inst = [None, None, None]
            for tg in range(3):
                for (r0, nr) in ROWCHUNKS:
                    cols = slice(r0 * W, (r0 + nr) * W)
                    p84 = pas.tile([28, RCH], F32, tag="px")
                    nc.tensor.matmul(p84[:, :nr * W], t_w84[:, tg, :],
                                     t_xs72[:, cols], start=True, stop=True)
                    t_xe = pa.tile([28, RCH], BF16, tag="txdbl", name="t_xe")
                    nc.scalar.copy(t_xe[:, :nr * W], p84[:, :nr * W])
                    for hh in range(2):
                        nc.sync.dma_start(
                            out=d_ri[tg][hh, :, r0 * W:(r0 + nr) * W],
                            in_=t_xe[hh * 14:(hh + 1) * 14, :nr * W])
                ar_inst[tg] = nc.gpsimd.collective_compute(
                    "AllReduce", OP.add, RG,
                    ins=[d_ri[tg][:]], outs=[d_ro[tg][:]])
            ar_v1 = nc.gpsimd.collective_compute(
                "AllReduce", OP.add, RG, ins=[d_v1i[:]], outs=[d_v1o[:]])

        # z recompute at my positions (independent; fills AR window)
        t_z = {}
        with tc.tile_pool(name="pz", bufs=2) as pz, \
             tc.tile_pool(name="pzp", bufs=2, space="PSUM") as pzp:
            for zi, (mod, ck) in enumerate(
                    (("vi", 0), ("vi", 1), ("ir", 0), ("ir", 1))):
                xt = t_xvc if mod == "vi" else t_xic
                p_z = pzp.tile([DM, PC], F32, tag="pz2")
                mmr(p_z[:], t_wz[:, zi, :], xt[:],
                    start=True, stop=True)
                t_e = pz.tile([DM, PC], F32, tag="ze")
                nc.scalar.activation(t_e[:], p_z[:], AF.Exp,
                                     bias=0.0, scale=-1.0)
                nc.vector.tensor_scalar_add(t_e[:], t_e[:], 1.0)
                t_r = pz.tile([DM, PC], F32, tag="zr")
                nc.vector.reciprocal(t_r[:], t_e[:])
                tz = big.tile([DM, PC], F32, tag=f"z{zi}", name=f"z{zi}")
                nc.vector.tensor_mul(tz[:], p_z[:], t_r[:])
                t_z[(mod, ck)] = tz

        # =========== PHASE B: scan middle (full-tile staging) ===========
        with tc.tile_pool(name="pb", bufs=2) as pb, \
             tc.tile_pool(name="pb2", bufs=2) as pb2, \
             tc.tile_pool(name="pbp", bufs=2, space="PSUM") as pbp, \
             tc.tile_pool(name="pby", bufs=2, space="PSUM") as pby:
            for t in range(3):
                segs = ([(0, L, t)] if t < 2 else
                        [(0, HW, 2), (HW, L, 3)])  # (start, end, k) tile cols
                yc0 = HW if t < 2 else 0            # y column span
                dro = d_ro[t]
                # ---- full-tile staged loads (DMA) ----
                t_rR = pb2.tile([RNK, L], BF16, tag="rR")
                for hh in range(2):
                    ld = nc.sync.dma_start(
                        out=t_rR[:, hh * HW:(hh + 1) * HW],
                        in_=dro[hh, 0:RNK, :])
                    add_dep_helper(ld.ins, ar_inst[t].ins,
                                   reason="rR after AR")
                # B replicated to lanes (lane = n*DSL + d) via broadcast DMA
                t_Brep = pb.tile([LANES, L], BF16, tag="Brep")
                for half in (0, 1):
                    ld = nc.sync.dma_start(
                        out=t_Brep[:, half * HW:(half + 1) * HW],
                        in_=dro[half, RNK:RNK + NST, :]
                        .unsqueeze(1).broadcast_to([NST, DSL, HW]))
                    add_dep_helper(ld.ins, ar_inst[t].ins,
                                   reason="Brep after AR")
                # xs replicated to lanes (4 plain copies per half)
                t_urep = pb.tile([LANES, L], BF16, tag="urep")
                for half in (0, 1):
                    o = XOFF[TMODS[t][half]]
                    for n in range(NST):
                        nc.sync.dma_start(
                            out=t_urep[n * DSL:(n + 1) * DSL,
                                       half * HW:(half + 1) * HW],
                            in_=t_xs72[o:o + DSL, :])
                # C replicated (y cols only)
                t_Crep = pb.tile([LANES, L], BF16, tag="Crep")
                for half in ((1,) if t < 2 else (0, 1)):
                    ld = nc.sync.dma_start(
                        out=t_Crep[:, half * HW:(half + 1) * HW],
                        in_=dro[half, RNK + NST:14, :]
                        .unsqueeze(1).broadcast_to([NST, DSL, HW]))
                    add_dep_helper(ld.ins, ar_inst[t].ins,
                                   reason="Crep after AR")

                # ---- dts chunks: matmul + Exp evac (exp table) ----
                t_et = pb2.tile([LANES, L], F32, tag="eta")
                for c in range(NCH):
                    c0 = c * CH
                    pieces = [(max(s, c0), min(e, c0 + CH), k)
                              for (s, e, k) in segs
                              if e > c0 and s < c0 + CH]
                    p_dts = pbp.tile([LANES, CH], F32, tag="dts")
                    for (s, e, k) in pieces:
                        nc.tensor.matmul(p_dts[:, s - c0:e - c0],
                                         t_wdtr[:, k, :],
                                         t_rR[:, s:e], start=True, stop=True)
                    for (s, e, k) in pieces:
                        nc.scalar.activation(t_et[:, s:e],
                                             p_dts[:, s - c0:e - c0], AF.Exp,
                                             bias=t_dtb[:, k:k + 1], scale=1.0)
                # ---- delta = softplus (ln table), then a = exp(asc*delta) ----
                t_delta = pb.tile([LANES, L], BF16, tag="delta")
                nc.scalar.activation(t_delta[:], t_et[:], AF.Ln,
                                     bias=1.0, scale=1.0)
                t_a = pb2.tile([LANES, L], F32, tag="eta")  # reuse et buffer
                for (s, e, k) in segs:
                    nc.scalar.activation(t_a[:, s:e], t_delta[:, s:e], AF.Exp,
                                         bias=0.0, scale=t_asc[:, k:k + 1])
                # ---- b = delta * B_rep * xs_rep (in-place, gpsimd + DVE) ----
                nc.gpsimd.tensor_mul(t_Brep[:], t_delta[:], t_Brep[:])
                nc.vector.tensor_mul(t_Brep[:], t_Brep[:], t_urep[:])
                # ---- scan ----
                t_h = pb.tile([LANES, L], F32, tag="h")
                if t < 2:
                    nc.vector.tensor_tensor_scan(t_h[:].bitcast(F32R), t_a[:],
                                                 t_Brep[:], 0.0,
                                                 OP.mult, OP.add)
                else:
                    for (s, e, k) in segs:   # reversed scans, fresh state
                        nc.vector.tensor_tensor_scan(
                            t_h[:, s:e][:, ::-1].bitcast(F32R),
                            t_a[:, s:e][:, ::-1],
                            t_Brep[:, s:e][:, ::-1], 0.0, OP.mult, OP.add)
                # ---- hc = h * C_rep (in-place into h), y cols only ----
                hc_eng = nc.gpsimd if t < 2 else nc.vector
                hc_eng.tensor_mul(t_h[:, yc0:L].bitcast(F32R),
                                  t_h[:, yc0:L], t_Crep[:, yc0:L])
                # ---- y = m96 @ hc (+ D skip) -> accumulate into yvi/yir ----
                c0 = yc0
                while c0 < L:
                    c1 = min(c0 + CH, L)
                    p_y = pby.tile([DSL, CH], F32, tag="y")
                    mmr(p_y[:, :c1 - c0], t_m96[:], t_h[:, c0:c1],
                        start=True, stop=(t == 2))
                    if t < 2:   # D-skip, combined (D_k + D_{k+2}) on fwd tiles
                        o = XOFF[TMODS[t][1]]
                        nc.tensor.matmul(
                            p_y[:, :c1 - c0], t_diagd[o:o + DSL, t, :],
                            t_xs72[o:o + DSL, c0 - HW:c1 - HW],
                            start=False, stop=True)
                        dst = t_yvi if t == 0 else t_yir
                        nc.scalar.copy(dst[:, c0 - HW:c1 - HW],
                                       p_y[:, :c1 - c0])
                    else:
                        for (s, e) in ((c0, min(c1, HW)), (max(c0, HW), c1)):
                            if e <= s:
                                continue
                            if e <= HW:
                                nc.vector.tensor_add(
                                    t_yvi[:, s:e], t_yvi[:, s:e],
                                    p_y[:, s - c0:e - c0])
                            else:
                                nc.vector.tensor_add(
                                    t_yir[:, s - HW:e - HW],
                                    t_yir[:, s - HW:e - HW],
                                    p_y[:, s - c0:e - c0])
                    c0 = c1

        # =========== A2A: reshard y channels -> positions ===========
        for j in range(NCORES):
            nc.sync.dma_start(out=d_a2i[j, 0:DSL, :],
                              in_=t_yvi[:, j * PC:(j + 1) * PC])
            nc.sync.dma_start(out=d_a2i[j, DSL:2 * DSL, :],
                              in_=t_yir[:, j * PC:(j + 1) * PC])
        a2a_inst = nc.gpsimd.collective_compute(
            "AllToAll", OP.bypass, RG, ins=[d_a2i[:]], outs=[d_a2o[:]])
        t_v1o = big.tile([12, 4], F32, tag="v1o")
        ld = nc.sync.dma_start(out=t_v1o[:], in_=d_v1o[:])
        add_dep_helper(ld.ins, ar_v1.ins, reason="v1 after AR")

        # =========== PHASE C: LN + gate + out (position-sharded) ===========
        with tc.tile_pool(name="pcq", bufs=2) as pcq, \
             tc.tile_pool(name="pcp", bufs=1, space="PSUM") as pcp:
            # gather y chunks [96, PC] x (2 chunks, 2 mods)
            t_y = {}
            for mod, roff in (("vi", 0), ("ir", DSL)):
                for ck in range(2):
                    ty = pcq.tile([DM, PC], F32, tag=f"y{mod}{ck}",
                                  name=f"y{mod}{ck}")
                    for jj in range(4):
                        j = ck * 4 + jj
                        ld = nc.sync.dma_start(
                            out=ty[jj * DSL:(jj + 1) * DSL, :].bitcast(F32R),
                            in_=d_a2o[j, roff:roff + DSL, :].bitcast(F32R))
                        add_dep_helper(ld.ins, a2a_inst.ins,
                                       reason="y after A2A")
                    t_y[(mod, ck)] = ty
            # chan-attn scales s = 1 + sigmoid(f2 @ (relu(va)+relu(vm)))
            t_vr = pcq.tile([12, 4], F32, tag="vr")
            nc.scalar.activation(t_vr[:], t_v1o[:], AF.Relu)
            t_vw = pcq.tile([12, 2], F32, tag="vw")
            nc.vector.tensor_add(t_vw[:, 0:1], t_vr[:, 0:1], t_vr[:, 1:2])
            nc.vector.tensor_add(t_vw[:, 1:2], t_vr[:, 2:3], t_vr[:, 3:4])
            t_s = {}
            for ck in range(2):
                p_ca = pcp.tile([DM, 2], F32, tag="pca")
                for mod_i in range(2):
                    nc.tensor.matmul(p_ca[:, mod_i:mod_i + 1],
                                     t_f2[:, mod_i, ck, :],
                                     t_vw[:, mod_i:mod_i + 1],
                                     start=True, stop=True)
                t_e = pcq.tile([DM, 2], F32, tag="cae")
                nc.scalar.activation(t_e[:], p_ca[:], AF.Exp,
                                     bias=0.0, scale=-1.0)
                nc.vector.tensor_scalar_add(t_e[:], t_e[:], 1.0)
                t_r = pcq.tile([DM, 2], F32, tag=f"car{ck}", name=f"car{ck}")
                nc.vector.reciprocal(t_r[:], t_e[:])          # sigmoid
                nc.vector.tensor_scalar_add(t_r[:], t_r[:], 1.0)  # 1+sigmoid
                t_s[ck] = t_r
            # LN per modality
            t_fin = {}
            for mod in ("vi", "ir"):
                p_s1 = pcp.tile([1, PC], F32, tag="s1")
                p_s2 = pcp.tile([1, PC], F32, tag="s2")
                for ck in range(2):
                    nc.tensor.matmul(p_s1[:], t_onec[:],
                                     t_y[(mod, ck)][:], start=(ck == 0),
                                     stop=(ck == 1))
                for ck in range(2):
                    t_sq = pcq.tile([DM, PC], F32, tag="sq")
                    nc.scalar.activation(t_sq[:], t_y[(mod, ck)][:], AF.Square)
                    nc.tensor.matmul(p_s2[:], t_onec[:],
                                     t_sq[:], start=(ck == 0), stop=(ck == 1))
                t_mu = pcq.tile([1, PC], F32, tag="mu")
                nc.vector.tensor_scalar_mul(t_mu[:], p_s1[:], 1.0 / DI)
                t_musq = pcq.tile([1, PC], F32, tag="musq")
                nc.vector.tensor_mul(t_musq[:], t_mu[:], t_mu[:])
                t_var = pcq.tile([1, PC], F32, tag="var")
                nc.vector.scalar_tensor_tensor(t_var[:], p_s2[:], 1.0 / DI,
                                               t_musq[:], OP.mult, OP.subtract)
                t_eps = pcq.tile([1, 1], F32, tag="eps")
                nc.vector.memset(t_eps[:], 1e-5)
                t_lnv = pcq.tile([1, PC], F32, tag="lnv")
                nc.scalar.activation(t_lnv[:], t_var[:], AF.Ln,
                                     bias=t_eps[:], scale=1.0)
                t_rstd = pcq.tile([1, PC], F32, tag="rstd")
                nc.scalar.activation(t_rstd[:], t_lnv[:], AF.Exp,
                                     bias=0.0, scale=-0.5)
                t_mur = pcq.tile([1, PC], F32, tag="mur")
                nc.vector.tensor_mul(t_mur[:], t_mu[:], t_rstd[:])
                p_q = pcp.tile([DM, PC], F32, tag="pq")
                nc.tensor.matmul(p_q[:], t_oner[:], t_rstd[:],
                                 start=True, stop=True)
                p_m = pcp.tile([DM, PC], F32, tag="pm")
                nc.tensor.matmul(p_m[:], t_oner[:], t_mur[:],
                                 start=True, stop=True)
                gb = {"vi": (0, 1), "ir": (2, 3)}[mod]
                for ck in range(2):
                    t_t = pcq.tile([DM, PC], F32, tag="lt")
                    nc.vector.tensor_mul(t_t[:], t_y[(mod, ck)][:], p_q[:])
                    t_t2 = pcq.tile([DM, PC], F32, tag="lt2")
                    nc.vector.tensor_sub(t_t2[:], t_t[:], p_m[:])
                    t_yn = pcq.tile([DM, PC], F32, tag="yn")
                    nc.scalar.activation(t_yn[:], t_t2[:], AF.Identity,
                                         bias=t_lnw[:, ck, gb[1]:gb[1] + 1],
                                         scale=t_lnw[:, ck, gb[0]:gb[0] + 1])
                    # gate: fin += yn * z * s
                    t_m1 = pcq.tile([DM, PC], F32, tag="m1")
                    nc.vector.tensor_mul(t_m1[:], t_yn[:], t_z[(mod, ck)][:])
                    if mod == "vi":
                        t_f = pcq.tile([DM, PC], F32, tag=f"fin{ck}",
                                       name=f"fin{ck}")
                        nc.vector.tensor_scalar_mul(t_f[:].bitcast(F32R),
                                                    t_m1[:],
                                                    t_s[ck][:, 0:1])
                        t_fin[ck] = t_f
                    else:
                        nc.vector.scalar_tensor_tensor(
                            t_fin[ck][:].bitcast(F32R), t_m1[:],
                            t_s[ck][:, 1:2], t_fin[ck][:],
                            OP.mult, OP.add)
            p_o = pcp.tile([DM, PC], F32, tag="po")
            for ck in range(2):
                mmr(p_o[:], t_wout[:, ck, :], t_fin[ck][:],
                    start=(ck == 0), stop=(ck == 1))
            t_o = pcq.tile([DM, PC], F32, tag="o")
            nc.scalar.copy(t_o[:], p_o[:])
            nc.sync.dma_start(out=o_out[:], in_=t_o[:])

    nc.finalize()
    return nc


def _prep_inputs(inputs):
    """Host-side prep: slice/transpose weights per core. Returns in_maps."""
    g = {k: np.asarray(v, dtype=np.float32) for k, v in inputs.items()}
    x_vi = g["x_vi"].reshape(HW, DM)
    x_ir = g["x_ir"].reshape(HW, DM)
    xvt = np.ascontiguousarray(x_vi.T)
    xit = np.ascontiguousarray(x_ir.T)
    A = -np.exp(g["A_logs"]).reshape(K, DI, NST)
    Ds = g["Ds"].reshape(K, DI)
    in_maps = []
    for c in range(NCORES):
        S = slice(c * DSL, (c + 1) * DSL)
        m = {}
        m["xvt"], m["xit"] = xvt, xit
        w48v = np.zeros((DM, 64), np.float32)
        w48v[:, 0:DSL] = g["W_vi"][S].T
        w48v[:, 32:56] = g["W_vi"][DI:][S].T
        m["w48v"] = w48v
        w48i = np.zeros((DM, 64), np.float32)
        w48i[:, 0:DSL] = g["W_ir"][S].T
        w48i[:, 32:56] = g["W_ir"][DI:][S].T
        m["w48i"] = w48i
        m["wsub"] = np.ascontiguousarray(g["W_sub"][S].T)
        w72 = np.zeros((96, 9, 96), np.float32)
        b72 = np.zeros((96, 1), np.float32)
        for nm in ("sub", "vi", "ir"):
            o = XOFF[nm]
            cw = g[f"conv_w_{nm}"][S, 0]      # [DSL, 3, 3]
            for tap in range(9):
                for d in range(DSL):
                    w72[o + d, tap, o + d] = cw[d, tap // 3, tap % 3]
            b72[o:o + DSL, 0] = g[f"conv_b_{nm}"][S]
        m["w72"], m["b72"] = w72, b72
        # x_dbl lhsT per scan tile: blocks (tile, half) -> (k, src mod)
        w84 = np.zeros((96, 3, 28), np.float32)
        BLK = (((0, "sub"), (0, "vi")), ((1, "sub"), (1, "ir")),
               ((2, "vi"), (3, "ir")))
        for tg in range(3):
            for half, (k, nm) in enumerate(BLK[tg]):
                o = XOFF[nm]
                w84[o:o + DSL, tg, half * 14:(half + 1) * 14] = \
                    g["x_proj_weight"][k][:, S].T
        m["w84"] = w84.astype(ml_dtypes.bfloat16)
        wdtr = np.zeros((RNK, K, LANES), np.float32)
        dtb = np.zeros((LANES, K), np.float32)
        asc = np.zeros((LANES, K), np.float32)
        for k in range(K):
            for n in range(NST):
                for d in range(DSL):
                    lane = n * DSL + d
                    wdtr[:, k, lane] = g["dt_projs_weight"][k, c * DSL + d, :]
                    dtb[lane, k] = g["dt_projs_bias"][k, c * DSL + d]
                    asc[lane, k] = A[k, c * DSL + d, n]
        m["wdtr"] = wdtr.astype(ml_dtypes.bfloat16)
        m["dtb"], m["asc"] = dtb, asc
        m96 = np.zeros((LANES, DSL), np.float32)
        for n in range(NST):
            for d in range(DSL):
                m96[n * DSL + d, d] = 1
        m["m96"] = m96
        diagd = np.zeros((96, 2, DSL), np.float32)
        np.fill_diagonal(diagd[XOFF["vi"]:XOFF["vi"] + DSL, 0, :],
                         Ds[0, S] + Ds[2, S])
        np.fill_diagonal(diagd[XOFF["ir"]:XOFF["ir"] + DSL, 1, :],
                         Ds[1, S] + Ds[3, S])
        m["diagd"] = diagd.astype(ml_dtypes.bfloat16)
        f1 = np.zeros((DSL, 4, 12), np.float32)
        f1[:, 0] = g["ca_vi_f1"][:, S].T / HW
        f1[:, 1] = g["ca_vi_f1"][:, S].T
        f1[:, 2] = g["ca_ir_f1"][:, S].T / HW
        f1[:, 3] = g["ca_ir_f1"][:, S].T
        m["f1"] = f1
        f2 = np.zeros((12, 2, 2, DM), np.float32)
        for ck in range(2):
            f2[:, 0, ck] = g["ca_vi_f2"][ck * DM:(ck + 1) * DM].T
            f2[:, 1, ck] = g["ca_ir_f2"][ck * DM:(ck + 1) * DM].T
        m["f2"] = f2
        lnw = np.zeros((DM, 2, 4), np.float32)
        for ck in range(2):
            cs = slice(ck * DM, (ck + 1) * DM)
            lnw[:, ck, 0] = g["ln_vi_g"][cs]
            lnw[:, ck, 1] = g["ln_vi_b"][cs]
            lnw[:, ck, 2] = g["ln_ir_g"][cs]
            lnw[:, ck, 3] = g["ln_ir_b"][cs]
        m["lnw"] = lnw
        wout = np.zeros((DM, 2, DM), np.float32)
        for ck in range(2):
            wout[:, ck] = g["W_out"][:, ck * DM:(ck + 1) * DM].T
        m["wout"] = wout
        wz = np.zeros((DM, 4, DM), np.float32)
        wz[:, 0] = g["W_vi"][DI:][0:DM].T
        wz[:, 1] = g["W_vi"][DI:][DM:DI].T
        wz[:, 2] = g["W_ir"][DI:][0:DM].T
        wz[:, 3] = g["W_ir"][DI:][DM:DI].T
        m["wz"] = wz
        m["onec"] = np.ones((DM, 1), np.float32)
        m["oner"] = np.ones((1, DM), np.float32)
        m["xvc"] = np.ascontiguousarray(xvt[:, c * PC:(c + 1) * PC])
        m["xic"] = np.ascontiguousarray(xit[:, c * PC:(c + 1) * PC])
        in_maps.append(m)
    return in_maps


def kernel(**inputs):
    if "nc" not in _cache:
        _cache["nc"] = _build()
    nc = _cache["nc"]
    in_maps = _prep_inputs(inputs)
    res = run_bass_kernel_spmd(nc, in_maps, core_ids=list(range(NCORES)))
    out = np.zeros((DM, HW), np.float32)
    for c in range(NCORES):
        out[:, c * PC:(c + 1) * PC] = res.results[c]["out"]
    return out.T.reshape(B, H, W, DM).astype(np.float32)
